# revision 18
# baseline (speedup 1.0000x reference)
"""Trainium2 Bass kernel: multi-head attention (transposed-causal softmax).

Reference math (B=4, N=2048, D=1024, H=16, E=64):
    qkv = x @ W_qkv -> split (3, H, E)
    scores[i, j] = k_i . q_j / sqrt(E)          (i = key pos, j = query pos)
    mask: keep i <= j; softmax over j; out[i] = sum_j attn[i, j] v_j
    y = concat_heads(out) @ W_o
Sharding (8 cores): data-parallel over batch (4) x tensor-parallel over
head-groups (2 groups of 8 heads); the host sums the two partial
projections per batch.

Per-core dataflow (v2):
  - xt [D, N] supplied transposed so projections contract D on partitions.
  - V is projected straight into natural layout (rows = positions) for all
    8 heads up front: vna [128, NT*8*65] with a ones column per head, so
    the AV matmul emits O^T rows plus the softmax denominator Z.
  - scores are built transposed per head pair (head A on partitions 0-63,
    head B on 64-127; the two K=64 matmuls occupy disjoint PE row groups
    and run concurrently).  exp on ScalarE covers both heads in one op.
  - The attention inner loop is the ScalarE-exp rate-limited phase, so PE
    filler work (next pair's Q/K projection chunks, output-projection
    tiles) is interleaved into it by emission order.
  - Z normalization: DVE evacuates PSUM O^T+Z to SBUF (frees the bank),
    reciprocal_approx_fast on the Z row, GPSIMD partition_broadcast, then
    two DVE muls write the normalized O^T; head B's rows reach SBUF
    partitions 64-127 via a small SBUF-to-SBUF DMA.
"""

import os
import sys
from collections import deque
from contextlib import ExitStack

import numpy as np

for _p in ("/opt/trn_rl_repo",):
    if os.path.isdir(_p) and _p not in sys.path:
        sys.path.insert(0, _p)

import ml_dtypes

import concourse.bacc as bacc
import concourse.mybir as mybir
import concourse.tile as tile
from concourse.bass_utils import run_bass_kernel_spmd
from concourse.masks import make_lower_triangular

AF = mybir.ActivationFunctionType
F32 = mybir.dt.float32
BF16 = mybir.dt.bfloat16
BF16NP = ml_dtypes.bfloat16

B, N, D, H, E = 4, 2048, 1024, 16, 64
N_CORES = 8
HPC = H // 2  # heads per core (tensor-parallel over 2 head groups)
CHW = 512     # i-chunk width (one fp32 PSUM bank)
VB = 65       # vna block per head: [V(64) | ones(1)]


def emit_attention(ctx, tc, y, xt, wq, wk, wv, wo, n, d, hpc, tri, dbg=None):
    nc = tc.nc
    KT = d // 128        # contraction tiles for projections
    NT = n // 128        # j-tiles
    NCH = n // CHW       # i-chunks
    NP = hpc // 2        # head pairs
    DQ = hpc * 64        # per-core q/k/v width
    OC = min(512, d)     # out-projection column chunk
    NOC = d // OC
    TPC = CHW // 128     # j-tiles per chunk width (4)
    DT = DQ // 128       # k-tiles for out projection

    # persistent SBUF tensors
    big = ctx.enter_context(tc.tile_pool(name="big", bufs=1))
    xt_sb = big.tile([128, KT * n], BF16, tag="xt", name="xt_sb")
    wq_sb = big.tile([128, KT * DQ], BF16, tag="wq", name="wq_sb")
    wk_sb = big.tile([128, KT * DQ], BF16, tag="wk", name="wk_sb")
    wv_sb = big.tile([128, KT * DQ], BF16, tag="wv", name="wv_sb")
    wo_sb = big.tile([128, DT * d], BF16, tag="wo", name="wo_sb")
    vna = big.tile([128, NT * hpc * VB], BF16, tag="vna", name="vna")
    ot_all = [big.tile([128, n], BF16, tag=f"ot{p_}", name=f"ot{p_}")
              for p_ in range(NP)]

    # input DMAs: wv + xt first (first consumers), then wq, wk, wo
    for k_ in range(KT):
        nc.sync.dma_start(
            wv_sb[:, k_ * DQ : (k_ + 1) * DQ], wv[k_ * 128 : (k_ + 1) * 128, :]
        )
        nc.sync.dma_start(
            xt_sb[:, k_ * n : (k_ + 1) * n], xt[k_ * 128 : (k_ + 1) * 128, :]
        )
    for w_sb, wd in ((wq_sb, wq), (wk_sb, wk)):
        for k_ in range(KT):
            nc.sync.dma_start(
                w_sb[:, k_ * DQ : (k_ + 1) * DQ], wd[k_ * 128 : (k_ + 1) * 128, :]
            )
    for t_ in range(DT):
        nc.sync.dma_start(
            wo_sb[:, t_ * d : (t_ + 1) * d], wo[t_ * 128 : (t_ + 1) * 128, :]
        )

    # ones columns of vna (col 64 of each per-head 65-block)
    nc.vector.memset(
        vna.rearrange("p (b c) -> p b c", c=VB)[:, :, 64:65], 1.0
    )

    # working pools
    qkvp = ctx.enter_context(tc.tile_pool(name="qkv", bufs=2))
    ptp = ctx.enter_context(tc.tile_pool(name="pt", bufs=4))
    stp = ctx.enter_context(tc.tile_pool(name="st", bufs=2))
    npool = ctx.enter_context(tc.tile_pool(name="nrm", bufs=2))
    ysp = ctx.enter_context(tc.tile_pool(name="yst", bufs=2))
    psb = ctx.enter_context(tc.tile_pool(name="psb", bufs=2, space="PSUM"))
    pss = ctx.enter_context(tc.tile_pool(name="pss", bufs=2, space="PSUM"))
    pso = ctx.enter_context(tc.tile_pool(name="pso", bufs=1, space="PSUM"))

    def emit_v_chunk(t_):
        """V natural layout for all 8 heads, j-tile t_ (plus ones col)."""
        ps = psb.tile([128, DQ], F32, tag="big", name="ps_v")
        for k_ in range(KT):
            nc.tensor.matmul(
                ps,
                lhsT=xt_sb[:, k_ * n + t_ * 128 : k_ * n + (t_ + 1) * 128],
                rhs=wv_sb[:, k_ * DQ : (k_ + 1) * DQ],
                start=(k_ == 0),
                stop=(k_ == KT - 1),
            )
        nc.vector.tensor_copy(
            vna[:, t_ * hpc * VB : (t_ + 1) * hpc * VB].rearrange(
                "p (h c) -> p h c", c=VB
            )[:, :, 0:64],
            ps.rearrange("p (h c) -> p h c", c=64),
        )

    def emit_qk_chunk(w_sb, p_, chn, dst):
        """One 512-wide n-chunk of the transposed Q or K projection."""
        ps = psb.tile([128, 512], F32, tag="big", name="ps_qk")
        for k_ in range(KT):
            nc.tensor.matmul(
                ps,
                lhsT=w_sb[:, k_ * DQ + p_ * 128 : k_ * DQ + (p_ + 1) * 128],
                rhs=xt_sb[:, k_ * n + chn * 512 : k_ * n + chn * 512 + 512],
                start=(k_ == 0),
                stop=(k_ == KT - 1),
            )
        nc.vector.tensor_copy(dst[:, chn * 512 : (chn + 1) * 512], ps)

    def emit_proj_itile(it):
        """Output projection for one 128-row i-tile."""
        ys = ysp.tile([128, d], F32, tag="y", name="ys")
        for hf in range(NOC):
            pf = psb.tile([128, OC], F32, tag="big", name="pf")
            for p_ in range(NP):
                nc.tensor.matmul(
                    pf,
                    lhsT=ot_all[p_][:, it * 128 : (it + 1) * 128],
                    rhs=wo_sb[:, p_ * d + hf * OC : p_ * d + hf * OC + OC],
                    start=(p_ == 0),
                    stop=(p_ == NP - 1),
                )
            nc.vector.tensor_copy(ys[:, hf * OC : (hf + 1) * OC], pf)
        nc.sync.dma_start(y[it * 128 : (it + 1) * 128, :], ys)

    filler = deque()

    def pump(k=1):
        for _ in range(min(k, len(filler))):
            filler.popleft()()

    # prelude: full q^T for pair 0, k^T chunk NCH-1, V j-tiles for chunk NCH-1
    qt = qkvp.tile([128, n], BF16, tag="qt", name="qt0")
    kt = qkvp.tile([128, n], BF16, tag="kt", name="kt0")
    for chn in range(n // 512):
        emit_qk_chunk(wq_sb, 0, chn, qt)
    emit_qk_chunk(wk_sb, 0, NCH - 1, kt)
    for t_ in range(NT - 1, NT - 1 - TPC, -1):
        emit_v_chunk(t_)

    for p_ in range(NP):
        if p_ < NP - 1:
            # queue next pair's projections as PE filler work
            qt_n = qkvp.tile([128, n], BF16, tag="qt", name=f"qt{p_ + 1}")
            kt_n = qkvp.tile([128, n], BF16, tag="kt", name=f"kt{p_ + 1}")
            for chn in range(n // 512):
                filler.append(
                    lambda c=chn, t=qt_n, pp=p_ + 1: emit_qk_chunk(wq_sb, pp, c, t)
                )
            for chn in range(n // 512):
                filler.append(
                    lambda c=chn, t=kt_n, pp=p_ + 1: emit_qk_chunk(wk_sb, pp, c, t)
                )
            qk_next = (qt_n, kt_n)

        psab_of = {}

        def emit_scores(cc, t_):
            o = 128 * t_ - CHW * cc
            w = min(CHW, o + 128)
            ps = pss.tile([128, 2 * CHW], F32, tag="sAB", name="psab")
            nc.tensor.matmul(
                ps[:, :w],
                lhsT=qt[0:64, t_ * 128 : (t_ + 1) * 128],
                rhs=kt[0:64, cc * CHW : cc * CHW + w],
                start=True,
                stop=True,
            )
            nc.tensor.matmul(
                ps[:, CHW : CHW + w],
                lhsT=qt[64:128, t_ * 128 : (t_ + 1) * 128],
                rhs=kt[64:128, cc * CHW : cc * CHW + w],
                start=True,
                stop=True,
            )
            psab_of[(cc, t_)] = ps

        iters = [
            (cc, t_)
            for cc in range(NCH - 1, -1, -1)
            for t_ in range(NT - 1, TPC * cc - 1, -1)
        ]
        po = None
        for i_, (cc, t_) in enumerate(iters):
            o = 128 * t_ - CHW * cc
            w = min(CHW, o + 128)
            first = t_ == NT - 1
            last = t_ == TPC * cc
            idx = NT - 1 - t_
            if first:
                po = pso.tile([65, 2 * CHW], F32, tag="po", name="po")
            if i_ == 0:
                emit_scores(cc, t_)
            psab = psab_of.pop((cc, t_))
            pab = ptp.tile([128, 2 * CHW], BF16, tag="pAB", name="pab")
            if w == CHW:
                nc.scalar.activation(pab, psab, AF.Exp)
            else:
                nc.scalar.activation(
                    pab.rearrange("p (a c) -> p a c", a=2)[:, :, 0:w],
                    psab.rearrange("p (a c) -> p a c", a=2)[:, :, 0:w],
                    AF.Exp,
                )
            # keep ScalarE fed: next iteration's scores go ahead of AV/fillers
            if i_ + 1 < len(iters):
                emit_scores(*iters[i_ + 1])
            if o < CHW:  # diagonal tile: keep i <= j within the block
                nc.vector.tensor_mul(
                    pab[:, o : o + 128], pab[:, o : o + 128], tri
                )
                nc.vector.tensor_mul(
                    pab[:, CHW + o : CHW + o + 128],
                    pab[:, CHW + o : CHW + o + 128],
                    tri,
                )
            # PE filler while ScalarE runs exp
            if p_ == 0 and cc > 0 and idx < TPC:
                # V j-tiles the next (wider) i-chunk will need
                emit_v_chunk(TPC * cc - 1 - idx)
            else:
                pump(1)
            if p_ == 0 and cc > 0 and idx == (NT - TPC * cc) // 2:
                emit_qk_chunk(wk_sb, 0, cc - 1, kt)
            nc.tensor.matmul(
                po[:, :w],
                lhsT=vna[:, (t_ * hpc + 2 * p_) * VB : (t_ * hpc + 2 * p_) * VB + VB],
                rhs=pab[:, :w],
                start=first,
                stop=last,
                skip_group_check=True,
            )
            nc.tensor.matmul(
                po[:, CHW : CHW + w],
                lhsT=vna[:, (t_ * hpc + 2 * p_ + 1) * VB : (t_ * hpc + 2 * p_ + 1) * VB + VB],
                rhs=pab[:, CHW : CHW + w],
                start=first,
                stop=last,
                skip_group_check=True,
            )
            if not last:
                continue
            # evacuate + normalize: O^T rows 0-63 (head A) / Z row 64
            sraw = stp.tile([65, 2 * CHW], F32, tag="sraw", name="sraw")
            nc.vector.tensor_copy(sraw, po)
            zrow = npool.tile([1, 2 * CHW], F32, tag="zrow", name="zrow")
            nc.sync.dma_start(zrow, sraw[64:65, :])  # partition 64 -> 0
            zinv = npool.tile([1, 2 * CHW], F32, tag="zinv", name="zinv")
            nc.vector.reciprocal_approx_fast(zinv, zrow)
            zb = npool.tile([64, 2 * CHW], F32, tag="zb", name="zb")
            nc.gpsimd.partition_broadcast(zb, zinv)
            nc.vector.tensor_mul(
                ot_all[p_][0:64, cc * CHW : (cc + 1) * CHW],
                sraw[0:64, 0:CHW],
                zb[:, 0:CHW],
            )
            stb = stp.tile([64, CHW], BF16, tag="stb", name="stb")
            nc.vector.tensor_mul(stb, sraw[0:64, CHW : 2 * CHW], zb[:, CHW : 2 * CHW])
            nc.sync.dma_start(
                ot_all[p_][64:128, cc * CHW : (cc + 1) * CHW], stb
            )
            if dbg is not None and p_ == 0 and cc == 0:
                nc.sync.dma_start(dbg["sraw0"], sraw)
                nc.sync.dma_start(dbg["zinv0"], zinv)
                nc.sync.dma_start(dbg["zb0"], zb)
            if p_ == NP - 1:
                # this i-chunk is now complete across all pairs
                for it in range(TPC * cc, TPC * (cc + 1)):
                    filler.append(lambda i=it: emit_proj_itile(i))

        if dbg is not None and p_ == 0:
            nc.sync.dma_start(dbg["qt0"], qt)
            nc.sync.dma_start(dbg["kt0"], kt)
            nc.sync.dma_start(dbg["vna"], vna)
            nc.sync.dma_start(dbg["ot0"], ot_all[0])
        if p_ < NP - 1:
            pump(len(filler))  # safety drain before the pair that needs them
            qt, kt = qk_next
    pump(len(filler))


def build_nc(n=N, d=D, hpc=HPC, num_devices=N_CORES, enable_asserts=False,
             reps=1, debug_outs=False):
    nc = bacc.Bacc(
        "TRN2",
        target_bir_lowering=False,
        debug=False,
        enable_asserts=enable_asserts,
        num_devices=num_devices,
    )
    dq = hpc * 64
    xt = nc.dram_tensor("xt", [d, n], BF16, kind="ExternalInput").ap()
    wq = nc.dram_tensor("wq", [d, dq], BF16, kind="ExternalInput").ap()
    wk = nc.dram_tensor("wk", [d, dq], BF16, kind="ExternalInput").ap()
    wv = nc.dram_tensor("wv", [d, dq], BF16, kind="ExternalInput").ap()
    wo = nc.dram_tensor("wo", [dq, d], BF16, kind="ExternalInput").ap()
    y = nc.dram_tensor("y", [n, d], F32, kind="ExternalOutput").ap()
    dbg = None
    if debug_outs:
        NT_ = n // 128
        dbg = {
            "sraw0": nc.dram_tensor("dbg_sraw0", [65, 2 * CHW], F32, kind="ExternalOutput").ap(),
            "zinv0": nc.dram_tensor("dbg_zinv0", [1, 2 * CHW], F32, kind="ExternalOutput").ap(),
            "zb0": nc.dram_tensor("dbg_zb0", [64, 2 * CHW], F32, kind="ExternalOutput").ap(),
            "qt0": nc.dram_tensor("dbg_qt0", [128, n], BF16, kind="ExternalOutput").ap(),
            "kt0": nc.dram_tensor("dbg_kt0", [128, n], BF16, kind="ExternalOutput").ap(),
            "vna": nc.dram_tensor("dbg_vna", [128, NT_ * HPC * VB], BF16, kind="ExternalOutput").ap(),
            "ot0": nc.dram_tensor("dbg_ot0", [128, n], BF16, kind="ExternalOutput").ap(),
        }
    with tile.TileContext(nc) as tc:
        with ExitStack() as cctx:
            cpool = cctx.enter_context(tc.tile_pool(name="consts", bufs=1))
            tri = cpool.tile([128, 128], BF16, tag="tri", name="tri")
            make_lower_triangular(nc, tri, val=1.0, diag=True)
            for _rep in range(reps):
                with ExitStack() as ctx:
                    emit_attention(ctx, tc, y, xt, wq, wk, wv, wo, n, d, hpc,
                                   tri, dbg=dbg)
    nc.compile()
    return nc


def make_in_maps(x, W_qkv, W_o):
    scale = np.float32(1.0 / np.sqrt(E))
    dq = HPC * 64
    in_maps = []
    for c in range(N_CORES):
        b, g = divmod(c, 2)
        in_maps.append(
            {
                "xt": np.ascontiguousarray(x[b].T).astype(BF16NP),
                "wq": (W_qkv[:, g * dq : (g + 1) * dq] * scale).astype(BF16NP),
                "wk": np.ascontiguousarray(
                    W_qkv[:, D + g * dq : D + (g + 1) * dq]
                ).astype(BF16NP),
                "wv": np.ascontiguousarray(
                    W_qkv[:, 2 * D + g * dq : 2 * D + (g + 1) * dq]
                ).astype(BF16NP),
                "wo": np.ascontiguousarray(W_o[g * dq : (g + 1) * dq, :]).astype(
                    BF16NP
                ),
            }
        )
    return in_maps


_NC_CACHE = {}


def kernel(x, W_qkv, W_o):
    x = np.asarray(x, dtype=np.float32)
    W_qkv = np.asarray(W_qkv, dtype=np.float32)
    W_o = np.asarray(W_o, dtype=np.float32)
    if "nc" not in _NC_CACHE:
        _NC_CACHE["nc"] = build_nc()
    in_maps = make_in_maps(x, W_qkv, W_o)
    res = run_bass_kernel_spmd(_NC_CACHE["nc"], in_maps, list(range(N_CORES)))
    ys = [np.asarray(res.results[i]["y"], dtype=np.float32) for i in range(N_CORES)]
    return np.stack([ys[2 * b] + ys[2 * b + 1] for b in range(B)])


# revision 32
# speedup vs baseline: 1.1389x; 1.1389x over previous
"""Trainium2 Bass kernel: multi-head attention (transposed-causal softmax).

Reference math (B=4, N=2048, D=1024, H=16, E=64):
    qkv = x @ W_qkv -> split (3, H, E)
    scores[i, j] = k_i . q_j / sqrt(E)          (i = key pos, j = query pos)
    mask: keep i <= j; softmax over j; out[i] = sum_j attn[i, j] v_j
    y = concat_heads(out) @ W_o
Sharding (8 cores): data-parallel over batch (4) x tensor-parallel over
head-groups (2 groups of 8 heads); the host sums the two partial
projections per batch.

Per-core dataflow (v2):
  - xt [D, N] supplied transposed so projections contract D on partitions.
  - V is projected straight into natural layout (rows = positions) for all
    8 heads up front: vna [128, NT*8*65] with a ones column per head, so
    the AV matmul emits O^T rows plus the softmax denominator Z.
  - scores are built transposed per head pair (head A on partitions 0-63,
    head B on 64-127; the two K=64 matmuls occupy disjoint PE row groups
    and run concurrently).  exp on ScalarE covers both heads in one op.
  - The attention inner loop is the ScalarE-exp rate-limited phase, so PE
    filler work (next pair's Q/K projection chunks, output-projection
    tiles) is interleaved into it by emission order.
  - Z normalization: DVE evacuates PSUM O^T+Z to SBUF (frees the bank),
    reciprocal_approx_fast on the Z row, GPSIMD partition_broadcast, then
    two DVE muls write the normalized O^T; head B's rows reach SBUF
    partitions 64-127 via a small SBUF-to-SBUF DMA.
"""

import os
import sys
from collections import deque
from contextlib import ExitStack

import numpy as np

for _p in ("/opt/trn_rl_repo",):
    if os.path.isdir(_p) and _p not in sys.path:
        sys.path.insert(0, _p)

import ml_dtypes

import concourse.bacc as bacc
import concourse.mybir as mybir
import concourse.tile as tile
from concourse.bass_utils import run_bass_kernel_spmd
from concourse.masks import make_identity, make_lower_triangular

AF = mybir.ActivationFunctionType
F32 = mybir.dt.float32
BF16 = mybir.dt.bfloat16
BF16NP = ml_dtypes.bfloat16

B, N, D, H, E = 4, 2048, 1024, 16, 64
N_CORES = 8
HPC = H // 2  # heads per core (tensor-parallel over 2 head groups)
CHW = 512     # i-chunk width (one fp32 PSUM bank)
VB = 65       # vna block per head: [V(64) | ones(1)]


def emit_attention(ctx, tc, y, xt, wq, wk, wv, wo, n, d, hpc, tri, ident,
                   dbg=None, phases=("attn", "norm", "proj")):
    nc = tc.nc
    KT = d // 128        # contraction tiles for projections
    NT = n // 128        # j-tiles
    NCH = n // CHW       # i-chunks
    NP = hpc // 2        # head pairs
    DQ = hpc * 64        # per-core q/k/v width
    OC = min(512, d)     # out-projection column chunk
    NOC = d // OC
    TPC = CHW // 128     # j-tiles per chunk width (4)
    DT = DQ // 128       # k-tiles for out projection

    # persistent SBUF tensors
    big = ctx.enter_context(tc.tile_pool(name="big", bufs=1))
    xt_sb = big.tile([128, KT * n], BF16, tag="xt", name="xt_sb")
    wq_sb = big.tile([128, KT * DQ], BF16, tag="wq", name="wq_sb")
    wk_sb = big.tile([128, KT * DQ], BF16, tag="wk", name="wk_sb")
    wv_sb = big.tile([128, KT * DQ], BF16, tag="wv", name="wv_sb")
    wo_sb = big.tile([128, DT * d], BF16, tag="wo", name="wo_sb")
    vna = big.tile([128, NT * hpc * VB], BF16, tag="vna", name="vna")
    ot_all = [big.tile([128, n], BF16, tag=f"ot{p_}", name=f"ot{p_}")
              for p_ in range(NP)]

    # input DMAs: wv + xt first (first consumers), then wq, wk, wo
    for k_ in range(KT):
        nc.sync.dma_start(
            wv_sb[:, k_ * DQ : (k_ + 1) * DQ], wv[k_ * 128 : (k_ + 1) * 128, :]
        )
        nc.sync.dma_start(
            xt_sb[:, k_ * n : (k_ + 1) * n], xt[k_ * 128 : (k_ + 1) * 128, :]
        )
    for w_sb, wd in ((wq_sb, wq), (wk_sb, wk)):
        for k_ in range(KT):
            nc.sync.dma_start(
                w_sb[:, k_ * DQ : (k_ + 1) * DQ], wd[k_ * 128 : (k_ + 1) * 128, :]
            )
    for t_ in range(DT):
        nc.sync.dma_start(
            wo_sb[:, t_ * d : (t_ + 1) * d], wo[t_ * 128 : (t_ + 1) * 128, :]
        )

    # ones columns of vna (col 64 of each per-head 65-block)
    nc.vector.memset(
        vna.rearrange("p (b c) -> p b c", c=VB)[:, :, 64:65], 1.0
    )

    # working pools
    qkvp = ctx.enter_context(tc.tile_pool(name="qkv", bufs=2))
    ptp = ctx.enter_context(tc.tile_pool(name="pt", bufs=4))
    stp = ctx.enter_context(tc.tile_pool(name="st", bufs=2))
    npool = ctx.enter_context(tc.tile_pool(name="nrm", bufs=2))
    ysp = ctx.enter_context(tc.tile_pool(name="yst", bufs=2))
    psb = ctx.enter_context(tc.tile_pool(name="psb", bufs=2, space="PSUM"))
    pss = ctx.enter_context(tc.tile_pool(name="pss", bufs=2, space="PSUM"))
    pso = ctx.enter_context(tc.tile_pool(name="pso", bufs=1, space="PSUM"))

    def emit_v_chunk(t_):
        """V natural layout for all 8 heads, j-tile t_ (plus ones col)."""
        ps = psb.tile([128, DQ], F32, tag="big", name="ps_v")
        for k_ in range(KT):
            nc.tensor.matmul(
                ps,
                lhsT=xt_sb[:, k_ * n + t_ * 128 : k_ * n + (t_ + 1) * 128],
                rhs=wv_sb[:, k_ * DQ : (k_ + 1) * DQ],
                start=(k_ == 0),
                stop=(k_ == KT - 1),
            )
        nc.vector.tensor_copy(
            vna[:, t_ * hpc * VB : (t_ + 1) * hpc * VB].rearrange(
                "p (h c) -> p h c", c=VB
            )[:, :, 0:64],
            ps.rearrange("p (h c) -> p h c", c=64),
        )

    def emit_qk_chunk(w_sb, p_, chn, dst):
        """One 512-wide n-chunk of the transposed Q or K projection."""
        ps = psb.tile([128, 512], F32, tag="big", name="ps_qk")
        for k_ in range(KT):
            nc.tensor.matmul(
                ps,
                lhsT=w_sb[:, k_ * DQ + p_ * 128 : k_ * DQ + (p_ + 1) * 128],
                rhs=xt_sb[:, k_ * n + chn * 512 : k_ * n + chn * 512 + 512],
                start=(k_ == 0),
                stop=(k_ == KT - 1),
            )
        nc.vector.tensor_copy(dst[:, chn * 512 : (chn + 1) * 512], ps)

    ys_of = {}

    def emit_proj_half(it, hf):
        """Half of the output projection for one 128-row i-tile."""
        if hf == 0:
            ys_of[it] = ysp.tile([128, d], F32, tag="y", name="ys")
        ys = ys_of[it]
        pf = psb.tile([128, OC], F32, tag="big", name="pf")
        for p_ in range(NP):
            nc.tensor.matmul(
                pf,
                lhsT=ot_all[p_][:, it * 128 : (it + 1) * 128],
                rhs=wo_sb[:, p_ * d + hf * OC : p_ * d + hf * OC + OC],
                start=(p_ == 0),
                stop=(p_ == NP - 1),
            )
        nc.scalar.copy(ys[:, hf * OC : (hf + 1) * OC], pf)
        if hf == NOC - 1:
            nc.sync.dma_start(y[it * 128 : (it + 1) * 128, :], ys)
            del ys_of[it]

    filler = deque()

    def pump(k=1):
        for _ in range(min(k, len(filler))):
            filler.popleft()()

    # prelude: full q^T for pair 0, k^T chunk NCH-1, V j-tiles for chunk NCH-1
    qt = qkvp.tile([128, n], BF16, tag="qt", name="qt0")
    kt = qkvp.tile([128, n], BF16, tag="kt", name="kt0")
    for chn in range(n // 512):
        emit_qk_chunk(wq_sb, 0, chn, qt)
    emit_qk_chunk(wk_sb, 0, NCH - 1, kt)
    for t_ in range(NT - 1, NT - 1 - TPC, -1):
        emit_v_chunk(t_)

    for p_ in range(NP):
        if p_ < NP - 1:
            # queue next pair's projections as PE filler work
            qt_n = qkvp.tile([128, n], BF16, tag="qt", name=f"qt{p_ + 1}")
            kt_n = qkvp.tile([128, n], BF16, tag="kt", name=f"kt{p_ + 1}")
            for chn in range(n // 512):
                filler.append(
                    lambda c=chn, t=qt_n, pp=p_ + 1: emit_qk_chunk(wq_sb, pp, c, t)
                )
            for chn in range(n // 512):
                filler.append(
                    lambda c=chn, t=kt_n, pp=p_ + 1: emit_qk_chunk(wk_sb, pp, c, t)
                )
            qk_next = (qt_n, kt_n)

        psab_of = {}

        def emit_scores(cc, t_):
            o = 128 * t_ - CHW * cc
            w = min(CHW, o + 128)
            diag = o < CHW
            ps = pss.tile([128, 2 * CHW], F32, tag="sAB", name="psab")
            nc.tensor.matmul(
                ps[:, :w],
                lhsT=qt[0:64, t_ * 128 : (t_ + 1) * 128],
                rhs=kt[0:64, cc * CHW : cc * CHW + w],
                start=True,
                stop=True,
            )
            nc.tensor.matmul(
                ps[:, CHW : CHW + w],
                lhsT=qt[64:128, t_ * 128 : (t_ + 1) * 128],
                rhs=kt[64:128, cc * CHW : cc * CHW + w],
                start=True,
                stop=True,
            )
            if diag:
                # accumulate -1e9 into the strictly-masked (i > j) elements
                # of the diagonal 128-block so exp() zeroes them
                nc.tensor.matmul(
                    ps[:, o : o + 128],
                    lhsT=tri,
                    rhs=ident,
                    start=False,
                    stop=True,
                    skip_group_check=True,
                )
                nc.tensor.matmul(
                    ps[:, CHW + o : CHW + o + 128],
                    lhsT=tri,
                    rhs=ident,
                    start=False,
                    stop=True,
                    skip_group_check=True,
                )
            psab_of[(cc, t_)] = ps

        iters = [
            (cc, t_)
            for cc in range(NCH - 1, -1, -1)
            for t_ in range(NT - 1, TPC * cc - 1, -1)
        ]
        po = None
        if "attn" not in phases:
            iters = []
            pump(len(filler))

        def make_av(cc, t_, pab):
            o = 128 * t_ - CHW * cc
            w = min(CHW, o + 128)
            first = t_ == NT - 1
            last = t_ == TPC * cc

            def av():
                nonlocal po
                if first:
                    po = pso.tile([65, 2 * CHW], F32, tag="po", name="po")
                nc.tensor.matmul(
                    po[:, :w],
                    lhsT=vna[:, (t_ * hpc + 2 * p_) * VB : (t_ * hpc + 2 * p_) * VB + VB],
                    rhs=pab[:, :w],
                    start=first,
                    stop=last,
                    skip_group_check=True,
                )
                nc.tensor.matmul(
                    po[:, CHW : CHW + w],
                    lhsT=vna[:, (t_ * hpc + 2 * p_ + 1) * VB : (t_ * hpc + 2 * p_ + 1) * VB + VB],
                    rhs=pab[:, CHW : CHW + w],
                    start=first,
                    stop=last,
                    skip_group_check=True,
                )

            return av

        pending_av = None
        for i_, (cc, t_) in enumerate(iters):
            o = 128 * t_ - CHW * cc
            w = min(CHW, o + 128)
            last = t_ == TPC * cc
            idx = NT - 1 - t_
            if i_ == 0:
                emit_scores(cc, t_)
            psab = psab_of.pop((cc, t_))
            pab = ptp.tile([128, 2 * CHW], BF16, tag="pAB", name="pab")
            if w == CHW:
                nc.scalar.activation(pab, psab, AF.Exp)
            else:
                nc.scalar.activation(
                    pab.rearrange("p (a c) -> p a c", a=2)[:, :, 0:w],
                    psab.rearrange("p (a c) -> p a c", a=2)[:, :, 0:w],
                    AF.Exp,
                )
            # keep ScalarE fed: next iteration's scores go ahead of AV/fillers
            if i_ + 1 < len(iters):
                emit_scores(*iters[i_ + 1])
            # PE filler while ScalarE runs exp
            if p_ == 0 and cc > 0 and idx < TPC:
                # V j-tiles the next (wider) i-chunk will need
                emit_v_chunk(TPC * cc - 1 - idx)
            else:
                pump(1)
            if p_ == 0 and cc > 0 and idx == (NT - TPC * cc) // 2:
                emit_qk_chunk(wk_sb, 0, cc - 1, kt)
            # AV runs one slot behind so it never waits on this slot's exp
            if pending_av is not None:
                pending_av()
            pending_av = make_av(cc, t_, pab)
            if not last:
                continue
            pending_av()
            pending_av = None
            if "norm" not in phases:
                continue
            # evacuate + normalize: O^T rows 0-63 (head A) / Z row 64
            sraw = stp.tile([65, 2 * CHW], F32, tag="sraw", name="sraw")
            nc.vector.tensor_copy(sraw, po)
            zrow = npool.tile([1, 2 * CHW], F32, tag="zrow", name="zrow")
            nc.sync.dma_start(zrow, sraw[64:65, :])  # partition 64 -> 0
            zinv = npool.tile([1, 2 * CHW], F32, tag="zinv", name="zinv")
            nc.vector.reciprocal_approx_fast(zinv, zrow)
            zb = npool.tile([64, 2 * CHW], F32, tag="zb", name="zb")
            nc.gpsimd.partition_broadcast(zb, zinv)
            nc.vector.tensor_mul(
                ot_all[p_][0:64, cc * CHW : (cc + 1) * CHW],
                sraw[0:64, 0:CHW],
                zb[:, 0:CHW],
            )
            stb = stp.tile([64, CHW], BF16, tag="stb", name="stb")
            nc.vector.tensor_mul(stb, sraw[0:64, CHW : 2 * CHW], zb[:, CHW : 2 * CHW])
            nc.sync.dma_start(
                ot_all[p_][64:128, cc * CHW : (cc + 1) * CHW], stb
            )
            if dbg is not None and p_ == 0 and cc == 0:
                nc.sync.dma_start(dbg["sraw0"], sraw)
                nc.sync.dma_start(dbg["zinv0"], zinv)
                nc.sync.dma_start(dbg["zb0"], zb)
            if p_ == NP - 1 and "proj" in phases:
                # this i-chunk is now complete across all pairs
                for it in range(TPC * cc, TPC * (cc + 1)):
                    for hf in range(NOC):
                        filler.append(
                            lambda i=it, h=hf: emit_proj_half(i, h)
                        )

        if dbg is not None and p_ == 0:
            nc.sync.dma_start(dbg["qt0"], qt)
            nc.sync.dma_start(dbg["kt0"], kt)
            nc.sync.dma_start(dbg["vna"], vna)
            nc.sync.dma_start(dbg["ot0"], ot_all[0])
        if p_ < NP - 1:
            pump(len(filler))  # safety drain before the pair that needs them
            qt, kt = qk_next
    pump(len(filler))


def build_nc(n=N, d=D, hpc=HPC, num_devices=N_CORES, enable_asserts=False,
             reps=1, debug_outs=False, phases=("attn", "norm", "proj")):
    nc = bacc.Bacc(
        "TRN2",
        target_bir_lowering=False,
        debug=False,
        enable_asserts=enable_asserts,
        num_devices=num_devices,
    )
    dq = hpc * 64
    xt = nc.dram_tensor("xt", [d, n], BF16, kind="ExternalInput").ap()
    wq = nc.dram_tensor("wq", [d, dq], BF16, kind="ExternalInput").ap()
    wk = nc.dram_tensor("wk", [d, dq], BF16, kind="ExternalInput").ap()
    wv = nc.dram_tensor("wv", [d, dq], BF16, kind="ExternalInput").ap()
    wo = nc.dram_tensor("wo", [dq, d], BF16, kind="ExternalInput").ap()
    y = nc.dram_tensor("y", [n, d], F32, kind="ExternalOutput").ap()
    dbg = None
    if debug_outs:
        NT_ = n // 128
        dbg = {
            "sraw0": nc.dram_tensor("dbg_sraw0", [65, 2 * CHW], F32, kind="ExternalOutput").ap(),
            "zinv0": nc.dram_tensor("dbg_zinv0", [1, 2 * CHW], F32, kind="ExternalOutput").ap(),
            "zb0": nc.dram_tensor("dbg_zb0", [64, 2 * CHW], F32, kind="ExternalOutput").ap(),
            "qt0": nc.dram_tensor("dbg_qt0", [128, n], BF16, kind="ExternalOutput").ap(),
            "kt0": nc.dram_tensor("dbg_kt0", [128, n], BF16, kind="ExternalOutput").ap(),
            "vna": nc.dram_tensor("dbg_vna", [128, NT_ * HPC * VB], BF16, kind="ExternalOutput").ap(),
            "ot0": nc.dram_tensor("dbg_ot0", [128, n], BF16, kind="ExternalOutput").ap(),
        }
    with tile.TileContext(nc) as tc:
        with ExitStack() as cctx:
            cpool = cctx.enter_context(tc.tile_pool(name="consts", bufs=1))
            # mask-add weights: (tri^T @ ident)[j, i] = -1e9 for i > j
            tri = cpool.tile([128, 128], BF16, tag="tri", name="tri")
            make_lower_triangular(nc, tri, val=-1e9, diag=False)
            ident = cpool.tile([128, 128], BF16, tag="ident", name="ident")
            make_identity(nc, ident)
            for _rep in range(reps):
                with ExitStack() as ctx:
                    emit_attention(ctx, tc, y, xt, wq, wk, wv, wo, n, d, hpc,
                                   tri, ident, dbg=dbg, phases=phases)
    nc.compile()
    return nc


def make_in_maps(x, W_qkv, W_o):
    scale = np.float32(1.0 / np.sqrt(E))
    dq = HPC * 64
    in_maps = []
    for c in range(N_CORES):
        b, g = divmod(c, 2)
        in_maps.append(
            {
                "xt": np.ascontiguousarray(x[b].T).astype(BF16NP),
                "wq": (W_qkv[:, g * dq : (g + 1) * dq] * scale).astype(BF16NP),
                "wk": np.ascontiguousarray(
                    W_qkv[:, D + g * dq : D + (g + 1) * dq]
                ).astype(BF16NP),
                "wv": np.ascontiguousarray(
                    W_qkv[:, 2 * D + g * dq : 2 * D + (g + 1) * dq]
                ).astype(BF16NP),
                "wo": np.ascontiguousarray(W_o[g * dq : (g + 1) * dq, :]).astype(
                    BF16NP
                ),
            }
        )
    return in_maps


_NC_CACHE = {}


def kernel(x, W_qkv, W_o):
    x = np.asarray(x, dtype=np.float32)
    W_qkv = np.asarray(W_qkv, dtype=np.float32)
    W_o = np.asarray(W_o, dtype=np.float32)
    if "nc" not in _NC_CACHE:
        _NC_CACHE["nc"] = build_nc()
    in_maps = make_in_maps(x, W_qkv, W_o)
    res = run_bass_kernel_spmd(_NC_CACHE["nc"], in_maps, list(range(N_CORES)))
    ys = [np.asarray(res.results[i]["y"], dtype=np.float32) for i in range(N_CORES)]
    return np.stack([ys[2 * b] + ys[2 * b + 1] for b in range(B)])


# revision 40
# speedup vs baseline: 1.5067x; 1.3230x over previous
"""Trainium2 Bass kernel: multi-head attention (transposed-causal softmax).

Reference math (B=4, N=2048, D=1024, H=16, E=64):
    qkv = x @ W_qkv -> split (3, H, E)
    scores[i, j] = k_i . q_j / sqrt(E)          (i = key pos, j = query pos)
    mask: keep i <= j; softmax over j; out[i] = sum_j attn[i, j] v_j
    y = concat_heads(out) @ W_o
Sharding (8 cores): data-parallel over batch (4) x tensor-parallel over
head-groups (2 groups of 8 heads); the host sums the two partial
projections per batch.

Per-core dataflow (v2):
  - xt [D, N] supplied transposed so projections contract D on partitions.
  - V is projected straight into natural layout (rows = positions) for all
    8 heads up front: vna [128, NT*8*65] with a ones column per head, so
    the AV matmul emits O^T rows plus the softmax denominator Z.
  - scores are built transposed per head pair (head A on partitions 0-63,
    head B on 64-127; the two K=64 matmuls occupy disjoint PE row groups
    and run concurrently).  exp on ScalarE covers both heads in one op.
  - The attention inner loop is the ScalarE-exp rate-limited phase, so PE
    filler work (next pair's Q/K projection chunks, output-projection
    tiles) is interleaved into it by emission order.
  - Z normalization: DVE evacuates PSUM O^T+Z to SBUF (frees the bank),
    reciprocal_approx_fast on the Z row, GPSIMD partition_broadcast, then
    two DVE muls write the normalized O^T; head B's rows reach SBUF
    partitions 64-127 via a small SBUF-to-SBUF DMA.
"""

import os
import sys
from collections import deque
from contextlib import ExitStack

import numpy as np

for _p in ("/opt/trn_rl_repo",):
    if os.path.isdir(_p) and _p not in sys.path:
        sys.path.insert(0, _p)

import ml_dtypes

import concourse.bacc as bacc
import concourse.mybir as mybir
import concourse.tile as tile
from concourse.bass_utils import run_bass_kernel_spmd
from concourse.masks import make_identity, make_lower_triangular

AF = mybir.ActivationFunctionType
F32 = mybir.dt.float32
BF16 = mybir.dt.bfloat16
BF16NP = ml_dtypes.bfloat16

B, N, D, H, E = 4, 2048, 1024, 16, 64
N_CORES = 8
HPC = H // 2  # heads per core (tensor-parallel over 2 head groups)
CHW = 512     # i-chunk width (one fp32 PSUM bank)
VB = 65       # vna block per head: [V(64) | ones(1)]


def emit_attention(ctx, tc, y, xt, wq, wk, wv, wo, n, d, hpc, tri, ident,
                   dbg=None, phases=("attn", "norm", "proj")):
    nc = tc.nc
    KT = d // 128        # contraction tiles for projections
    NT = n // 128        # j-tiles
    NCH = n // CHW       # i-chunks
    NP = hpc // 2        # head pairs
    DQ = hpc * 64        # per-core q/k/v width
    OC = min(512, d)     # out-projection column chunk
    NOC = d // OC
    TPC = CHW // 128     # j-tiles per chunk width (4)
    DT = DQ // 128       # k-tiles for out projection

    # persistent SBUF tensors
    big = ctx.enter_context(tc.tile_pool(name="big", bufs=1))
    xt_sb = big.tile([128, KT * n], BF16, tag="xt", name="xt_sb")
    wq_sb = big.tile([128, KT * DQ], BF16, tag="wq", name="wq_sb")
    wk_sb = big.tile([128, KT * DQ], BF16, tag="wk", name="wk_sb")
    wv_sb = big.tile([128, KT * DQ], BF16, tag="wv", name="wv_sb")
    wo_sb = big.tile([128, DT * d], BF16, tag="wo", name="wo_sb")
    vna = big.tile([128, NT * hpc * VB], BF16, tag="vna", name="vna")
    ot_all = [big.tile([128, n], BF16, tag=f"ot{p_}", name=f"ot{p_}")
              for p_ in range(NP)]

    # input DMAs: wv + xt first (first consumers), then wq, wk, wo
    for k_ in range(KT):
        nc.sync.dma_start(
            wv_sb[:, k_ * DQ : (k_ + 1) * DQ], wv[k_ * 128 : (k_ + 1) * 128, :]
        )
        nc.sync.dma_start(
            xt_sb[:, k_ * n : (k_ + 1) * n], xt[k_ * 128 : (k_ + 1) * 128, :]
        )
    for w_sb, wd in ((wq_sb, wq), (wk_sb, wk)):
        for k_ in range(KT):
            nc.sync.dma_start(
                w_sb[:, k_ * DQ : (k_ + 1) * DQ], wd[k_ * 128 : (k_ + 1) * 128, :]
            )
    for t_ in range(DT):
        nc.sync.dma_start(
            wo_sb[:, t_ * d : (t_ + 1) * d], wo[t_ * 128 : (t_ + 1) * 128, :]
        )

    # ones columns of vna (col 64 of each per-head 65-block)
    nc.vector.memset(
        vna.rearrange("p (b c) -> p b c", c=VB)[:, :, 64:65], 1.0
    )

    # working pools
    qkvp = ctx.enter_context(tc.tile_pool(name="qkv", bufs=2))
    ptp = ctx.enter_context(tc.tile_pool(name="pt", bufs=4))
    stp = ctx.enter_context(tc.tile_pool(name="st", bufs=2))
    npool = ctx.enter_context(tc.tile_pool(name="nrm", bufs=2))
    ysp = ctx.enter_context(tc.tile_pool(name="yst", bufs=2))
    psb = ctx.enter_context(tc.tile_pool(name="psb", bufs=2, space="PSUM"))
    pss = ctx.enter_context(tc.tile_pool(name="pss", bufs=2, space="PSUM"))
    pso = ctx.enter_context(tc.tile_pool(name="pso", bufs=1, space="PSUM"))

    def emit_v_chunk(t_):
        """V natural layout for all 8 heads, j-tile t_ (plus ones col)."""
        ps = psb.tile([128, DQ], F32, tag="big", name="ps_v")
        for k_ in range(KT):
            nc.tensor.matmul(
                ps,
                lhsT=xt_sb[:, k_ * n + t_ * 128 : k_ * n + (t_ + 1) * 128],
                rhs=wv_sb[:, k_ * DQ : (k_ + 1) * DQ],
                start=(k_ == 0),
                stop=(k_ == KT - 1),
            )
        nc.vector.tensor_copy(
            vna[:, t_ * hpc * VB : (t_ + 1) * hpc * VB].rearrange(
                "p (h c) -> p h c", c=VB
            )[:, :, 0:64],
            ps.rearrange("p (h c) -> p h c", c=64),
        )

    def emit_qk_chunk(w_sb, p_, chn, dst):
        """One 512-wide n-chunk of the transposed Q or K projection."""
        ps = psb.tile([128, 512], F32, tag="big", name="ps_qk")
        for k_ in range(KT):
            nc.tensor.matmul(
                ps,
                lhsT=w_sb[:, k_ * DQ + p_ * 128 : k_ * DQ + (p_ + 1) * 128],
                rhs=xt_sb[:, k_ * n + chn * 512 : k_ * n + chn * 512 + 512],
                start=(k_ == 0),
                stop=(k_ == KT - 1),
            )
        nc.vector.tensor_copy(dst[:, chn * 512 : (chn + 1) * 512], ps)

    ys_of = {}

    def emit_proj_half(it, hf):
        """Half of the output projection for one 128-row i-tile."""
        if hf == 0:
            ys_of[it] = ysp.tile([128, d], F32, tag="y", name="ys")
        ys = ys_of[it]
        pf = psb.tile([128, OC], F32, tag="big", name="pf")
        for p_ in range(NP):
            nc.tensor.matmul(
                pf,
                lhsT=ot_all[p_][:, it * 128 : (it + 1) * 128],
                rhs=wo_sb[:, p_ * d + hf * OC : p_ * d + hf * OC + OC],
                start=(p_ == 0),
                stop=(p_ == NP - 1),
            )
        nc.scalar.copy(ys[:, hf * OC : (hf + 1) * OC], pf)
        if hf == NOC - 1:
            nc.sync.dma_start(y[it * 128 : (it + 1) * 128, :], ys)
            del ys_of[it]

    filler = deque()

    def pump(k=1):
        for _ in range(min(k, len(filler))):
            filler.popleft()()

    # prelude: full q^T for pair 0, k^T chunk NCH-1, V j-tiles for chunk NCH-1
    qt = qkvp.tile([128, n], BF16, tag="qt", name="qt0")
    kt = qkvp.tile([128, n], BF16, tag="kt", name="kt0")
    for chn in range(n // 512):
        emit_qk_chunk(wq_sb, 0, chn, qt)
    emit_qk_chunk(wk_sb, 0, NCH - 1, kt)
    for t_ in range(NT - 1, NT - 1 - TPC, -1):
        emit_v_chunk(t_)

    for p_ in range(NP):
        if p_ < NP - 1:
            # queue next pair's projections as PE filler work
            qt_n = qkvp.tile([128, n], BF16, tag="qt", name=f"qt{p_ + 1}")
            kt_n = qkvp.tile([128, n], BF16, tag="kt", name=f"kt{p_ + 1}")
            for chn in range(n // 512):
                filler.append(
                    lambda c=chn, t=qt_n, pp=p_ + 1: emit_qk_chunk(wq_sb, pp, c, t)
                )
            for chn in range(n // 512):
                filler.append(
                    lambda c=chn, t=kt_n, pp=p_ + 1: emit_qk_chunk(wk_sb, pp, c, t)
                )
            qk_next = (qt_n, kt_n)

        psab_of = {}

        def emit_scores(cc, t_):
            o = 128 * t_ - CHW * cc
            w = min(CHW, o + 128)
            diag = o < CHW
            ps = pss.tile([128, 2 * CHW], F32, tag="sAB", name="psab")
            nc.tensor.matmul(
                ps[:, :w],
                lhsT=qt[0:64, t_ * 128 : (t_ + 1) * 128],
                rhs=kt[0:64, cc * CHW : cc * CHW + w],
                start=True,
                stop=True,
            )
            nc.tensor.matmul(
                ps[:, CHW : CHW + w],
                lhsT=qt[64:128, t_ * 128 : (t_ + 1) * 128],
                rhs=kt[64:128, cc * CHW : cc * CHW + w],
                start=True,
                stop=True,
            )
            if diag:
                # accumulate -1e9 into the strictly-masked (i > j) elements
                # of the diagonal 128-block so exp() zeroes them
                nc.tensor.matmul(
                    ps[:, o : o + 128],
                    lhsT=tri,
                    rhs=ident,
                    start=False,
                    stop=True,
                    skip_group_check=True,
                )
                nc.tensor.matmul(
                    ps[:, CHW + o : CHW + o + 128],
                    lhsT=tri,
                    rhs=ident,
                    start=False,
                    stop=True,
                    skip_group_check=True,
                )
            psab_of[(cc, t_)] = ps

        iters = [
            (cc, t_)
            for cc in range(NCH - 1, -1, -1)
            for t_ in range(NT - 1, TPC * cc - 1, -1)
        ]
        po = None
        if "attn" not in phases:
            iters = []
            pump(len(filler))

        def make_av(cc, t_, pab):
            o = 128 * t_ - CHW * cc
            w = min(CHW, o + 128)
            first = t_ == NT - 1
            last = t_ == TPC * cc

            def av():
                nonlocal po
                if first:
                    po = pso.tile([65, 2 * CHW], F32, tag="po", name="po")
                nc.tensor.matmul(
                    po[:, :w],
                    lhsT=vna[:, (t_ * hpc + 2 * p_) * VB : (t_ * hpc + 2 * p_) * VB + VB],
                    rhs=pab[:, :w],
                    start=first,
                    stop=last,
                    skip_group_check=True,
                )
                nc.tensor.matmul(
                    po[:, CHW : CHW + w],
                    lhsT=vna[:, (t_ * hpc + 2 * p_ + 1) * VB : (t_ * hpc + 2 * p_ + 1) * VB + VB],
                    rhs=pab[:, CHW : CHW + w],
                    start=first,
                    stop=last,
                    skip_group_check=True,
                )

            return av

        pending_av = None
        for i_, (cc, t_) in enumerate(iters):
            o = 128 * t_ - CHW * cc
            w = min(CHW, o + 128)
            last = t_ == TPC * cc
            idx = NT - 1 - t_
            if i_ == 0:
                emit_scores(cc, t_)
            psab = psab_of.pop((cc, t_))
            pab = ptp.tile([128, 2 * CHW], BF16, tag="pAB", name="pab")
            if w == CHW:
                nc.scalar.activation(pab, psab, AF.Exp)
            else:
                nc.scalar.activation(
                    pab.rearrange("p (a c) -> p a c", a=2)[:, :, 0:w],
                    psab.rearrange("p (a c) -> p a c", a=2)[:, :, 0:w],
                    AF.Exp,
                )
            # keep ScalarE fed: next iteration's scores go ahead of AV/fillers
            if i_ + 1 < len(iters):
                emit_scores(*iters[i_ + 1])
            # PE filler while ScalarE runs exp
            if p_ == 0 and cc > 0 and idx < TPC:
                # V j-tiles the next (wider) i-chunk will need
                emit_v_chunk(TPC * cc - 1 - idx)
            else:
                pump(1)
            if p_ == 0 and cc > 0 and idx == (NT - TPC * cc) // 2:
                emit_qk_chunk(wk_sb, 0, cc - 1, kt)
            # AV runs one slot behind so it never waits on this slot's exp
            if pending_av is not None:
                pending_av()
            pending_av = make_av(cc, t_, pab)
            if not last:
                continue
            pending_av()
            pending_av = None
            if "norm" not in phases:
                continue
            # evacuate + normalize: O^T rows 0-63 (head A) / Z row 64
            sraw = stp.tile([65, 2 * CHW], F32, tag="sraw", name="sraw")
            nc.vector.tensor_copy(sraw, po)
            zrow = npool.tile([1, 2 * CHW], F32, tag="zrow", name="zrow")
            nc.sync.dma_start(zrow, sraw[64:65, :])  # partition 64 -> 0
            zinv = npool.tile([1, 2 * CHW], F32, tag="zinv", name="zinv")
            nc.vector.reciprocal_approx_fast(zinv, zrow)
            zb = npool.tile([64, 2 * CHW], F32, tag="zb", name="zb")
            nc.gpsimd.partition_broadcast(zb, zinv)
            nc.vector.tensor_mul(
                ot_all[p_][0:64, cc * CHW : (cc + 1) * CHW],
                sraw[0:64, 0:CHW],
                zb[:, 0:CHW],
            )
            stb = stp.tile([64, CHW], BF16, tag="stb", name="stb")
            nc.vector.tensor_mul(stb, sraw[0:64, CHW : 2 * CHW], zb[:, CHW : 2 * CHW])
            nc.sync.dma_start(
                ot_all[p_][64:128, cc * CHW : (cc + 1) * CHW], stb
            )
            if dbg is not None and p_ == 0 and cc == 0:
                nc.sync.dma_start(dbg["sraw0"], sraw)
                nc.sync.dma_start(dbg["zinv0"], zinv)
                nc.sync.dma_start(dbg["zb0"], zb)
            if p_ == NP - 1 and "proj" in phases:
                # this i-chunk is now complete across all pairs
                for it in range(TPC * cc, TPC * (cc + 1)):
                    for hf in range(NOC):
                        filler.append(
                            lambda i=it, h=hf: emit_proj_half(i, h)
                        )

        if dbg is not None and p_ == 0:
            nc.sync.dma_start(dbg["qt0"], qt)
            nc.sync.dma_start(dbg["kt0"], kt)
            nc.sync.dma_start(dbg["vna"], vna)
            nc.sync.dma_start(dbg["ot0"], ot_all[0])
        if p_ < NP - 1:
            pump(len(filler))  # safety drain before the pair that needs them
            qt, kt = qk_next
    pump(len(filler))


def build_nc(n=N, d=D, hpc=HPC, num_devices=N_CORES, enable_asserts=False,
             reps=1, debug_outs=False, phases=("attn", "norm", "proj")):
    nc = bacc.Bacc(
        "TRN2",
        target_bir_lowering=False,
        debug=False,
        enable_asserts=enable_asserts,
        num_devices=num_devices,
    )
    dq = hpc * 64
    xt = nc.dram_tensor("xt", [d, n], BF16, kind="ExternalInput").ap()
    wq = nc.dram_tensor("wq", [d, dq], BF16, kind="ExternalInput").ap()
    wk = nc.dram_tensor("wk", [d, dq], BF16, kind="ExternalInput").ap()
    wv = nc.dram_tensor("wv", [d, dq], BF16, kind="ExternalInput").ap()
    wo = nc.dram_tensor("wo", [dq, d], BF16, kind="ExternalInput").ap()
    y = nc.dram_tensor("y", [n, d], F32, kind="ExternalOutput").ap()
    dbg = None
    if debug_outs:
        NT_ = n // 128
        dbg = {
            "sraw0": nc.dram_tensor("dbg_sraw0", [65, 2 * CHW], F32, kind="ExternalOutput").ap(),
            "zinv0": nc.dram_tensor("dbg_zinv0", [1, 2 * CHW], F32, kind="ExternalOutput").ap(),
            "zb0": nc.dram_tensor("dbg_zb0", [64, 2 * CHW], F32, kind="ExternalOutput").ap(),
            "qt0": nc.dram_tensor("dbg_qt0", [128, n], BF16, kind="ExternalOutput").ap(),
            "kt0": nc.dram_tensor("dbg_kt0", [128, n], BF16, kind="ExternalOutput").ap(),
            "vna": nc.dram_tensor("dbg_vna", [128, NT_ * HPC * VB], BF16, kind="ExternalOutput").ap(),
            "ot0": nc.dram_tensor("dbg_ot0", [128, n], BF16, kind="ExternalOutput").ap(),
        }
    with tile.TileContext(nc) as tc:
        with ExitStack() as cctx:
            cpool = cctx.enter_context(tc.tile_pool(name="consts", bufs=1))
            # mask-add weights: (tri^T @ ident)[j, i] = -1e9 for i > j
            tri = cpool.tile([128, 128], BF16, tag="tri", name="tri")
            make_lower_triangular(nc, tri, val=-1e9, diag=False)
            ident = cpool.tile([128, 128], BF16, tag="ident", name="ident")
            make_identity(nc, ident)
            for _rep in range(reps):
                with ExitStack() as ctx:
                    emit_attention(ctx, tc, y, xt, wq, wk, wv, wo, n, d, hpc,
                                   tri, ident, dbg=dbg, phases=phases)
    nc.compile()
    return nc


def make_in_maps(x, W_qkv, W_o):
    scale = np.float32(1.0 / np.sqrt(E))
    dq = HPC * 64
    in_maps = []
    for c in range(N_CORES):
        b, g = divmod(c, 2)
        in_maps.append(
            {
                "xt": np.ascontiguousarray(x[b].T).astype(BF16NP),
                "wq": (W_qkv[:, g * dq : (g + 1) * dq] * scale).astype(BF16NP),
                "wk": np.ascontiguousarray(
                    W_qkv[:, D + g * dq : D + (g + 1) * dq]
                ).astype(BF16NP),
                "wv": np.ascontiguousarray(
                    W_qkv[:, 2 * D + g * dq : 2 * D + (g + 1) * dq]
                ).astype(BF16NP),
                "wo": np.ascontiguousarray(W_o[g * dq : (g + 1) * dq, :]).astype(
                    BF16NP
                ),
            }
        )
    return in_maps


_NC_CACHE = {}


def kernel(x, W_qkv, W_o):
    x = np.asarray(x, dtype=np.float32)
    W_qkv = np.asarray(W_qkv, dtype=np.float32)
    W_o = np.asarray(W_o, dtype=np.float32)
    if "nc" not in _NC_CACHE:
        _NC_CACHE["nc"] = build_nc()
    in_maps = make_in_maps(x, W_qkv, W_o)
    res = run_bass_kernel_spmd(_NC_CACHE["nc"], in_maps, list(range(N_CORES)))
    ys = [np.asarray(res.results[i]["y"], dtype=np.float32) for i in range(N_CORES)]
    return np.stack([ys[2 * b] + ys[2 * b + 1] for b in range(B)])
